# revision 17
# baseline (speedup 1.0000x reference)
"""Pairwise squared-Euclidean distance kernel for TRN2 (8 NeuronCores).

Problem: matrix_1 [8, 2048, 256] fp32 -> out [8, 2048, 2048] fp32 with
  out[b,i,j] = max(||x_i||^2 + ||x_j||^2 - 2 x_i.x_j, 0)

Sharding: data-parallel over batch; core b handles matrix_1[b] entirely.

Host-side prep (per core, <0.1% of total FLOPs):
  xt [256, 2048] fp8(e4m3) = x.T   (both matmul operands)
  ni [128, 16]   fp32  row norms of the quantized x (per-partition bias)

Device plan per 128-row block i:
  4 fp8 DoubleRow matmuls (K=256 in ONE pass each; the two 128-row
  contraction chunks ride the [128, 2, n] interleaved APs) -> psum = G
  psA (banks 0-1) -> ACT:  dA = Identity(-2*psA + ni)       [fp16]
  psB (banks 2-3) -> DVE:  dB = (psB * -2) + ni  (2-scalar) [fp16]
  (separate tiles: two readers of one PSUM tile get serialized by Tile)
  2 stores of 256 KiB -> out rows

Host then finishes: out = max(out + nj, 0) in fp32. fp8 inputs cost
~2.5e-3 norm-rel (vs the 2e-2 gate); fp16 output adds ~3e-4. DoubleRow
halves the PE streaming passes (4 matmuls/block, not 8), and the split
ACT/DVE readers drain PSUM concurrently -- together they cut the block
cadence below the single-reader (1.9us) and 8-matmul (1.8us) floors.
~32 junk matmuls warm the PE's HAM clock gate during the input-DMA wait.
"""

import numpy as np

import concourse.bass as bass
import concourse.mybir as mybir
from concourse import bacc, tile
from concourse.bass_utils import run_bass_kernel_spmd

B, S, R = 8, 2048, 256
P = 128            # SBUF partitions
NT = S // P        # 16 row blocks
NBW = 512          # matmul moving-dim block = one fp32 PSUM bank
NB = S // NBW      # 4 col blocks
KH = R // P        # 2 contraction chunks
HALF = S // 2      # ACT/DVE reader split point
NWARM = 18         # HAM warm-up matmuls

F32 = mybir.dt.float32
F16 = mybir.dt.float16
F8 = mybir.dt.float8e4


def build_nc():
    # Bacc (not plain Bass): its compile() runs move_matmul_waits_to_ldweights
    # + generate_event_semaphores, without which walrus rejects matmuls that
    # accumulated >1 semaphore wait ("Too many sync wait commands").
    nc = bacc.Bacc()
    xt = nc.declare_dram_parameter("xt", [P, KH, S], F8, isOutput=False)
    ni_in = nc.declare_dram_parameter("ni", [P, NT], F32, isOutput=False)
    out = nc.declare_dram_parameter("out", [S, S], F16, isOutput=True)

    with tile.TileContext(nc) as tc:
        with (
            tc.tile_pool(name="const", bufs=1) as cpool,
            tc.tile_pool(name="xt", bufs=1) as xt_pool,
            tc.tile_pool(name="nrm", bufs=1) as nrm_pool,
            tc.tile_pool(name="obufA", bufs=4) as oA_pool,
            tc.tile_pool(name="obufB", bufs=4) as oB_pool,
            tc.tile_pool(name="psumA", bufs=2, space="PSUM") as psumA_pool,
            tc.tile_pool(name="psumB", bufs=2, space="PSUM") as psumB_pool,
        ):
            # HAM warm-up inputs: only need their memsets, no DMA dependency.
            junkw = cpool.tile([P, P], F16)
            junkm = cpool.tile([P, P], F16)
            nc.gpsimd.memset(junkw[:], 0.0)
            nc.gpsimd.memset(junkm[:], 0.0)

            # Both contraction chunks in one tile so a single [128, 2, n]
            # AP feeds DoubleRow: XTD[:, k, j] = x[j, 128k + p].
            XTD = xt_pool.tile([P, KH, S], F8)
            NI = nrm_pool.tile([P, NT], F32)

            nc.sync.dma_start(NI[:], ni_in[:, :])
            # One 512 KiB load: the host uploads xt already in the tile's
            # [p, k, j] layout, so this is 4 KiB contiguous per partition --
            # far better descriptors than eight 64 KiB column chunks.
            nc.sync.dma_start(XTD[:], xt[:, :, :])

            # Junk matmuls keep the PE continuously busy from t~6.5us until
            # the first data chunks land, so HAM unthrottles to 2.4 GHz.
            warmp = psumA_pool.tile([P, HALF], F32, tag="psA")
            for w in range(NWARM):
                nc.tensor.matmul(warmp[:, (w % 8) * P:(w % 8 + 1) * P],
                                 junkw[:], junkm[:], start=True, stop=True)

            # --- main loop over row blocks ---
            for i in range(NT):
                isl = slice(i * P, (i + 1) * P)
                psA = psumA_pool.tile([P, HALF], F32, tag="psA")
                psB = psumB_pool.tile([P, HALF], F32, tag="psB")
                lhsT = XTD[:, :, isl]
                for j in range(NB):
                    jsl = slice(j * NBW, (j + 1) * NBW)
                    dst = psA[:, jsl] if j < NB // 2 else \
                        psB[:, j * NBW - HALF:(j + 1) * NBW - HALF]
                    nc.tensor.matmul(
                        dst, lhsT, XTD[:, :, jsl],
                        start=True, stop=True,
                        perf_mode=mybir.MatmulPerfMode.DoubleRow,
                    )
                dA = oA_pool.tile([P, HALF], F16, tag="dA")
                dB = oB_pool.tile([P, HALF], F16, tag="dB")
                nc.scalar.activation(
                    dA[:], psA[:],
                    mybir.ActivationFunctionType.Identity,
                    bias=NI[:, i:i + 1], scale=-2.0,
                )
                nc.vector.tensor_scalar(
                    out=dB[:], in0=psB[:],
                    scalar1=-2.0, scalar2=NI[:, i:i + 1],
                    op0=mybir.AluOpType.mult, op1=mybir.AluOpType.add,
                )
                nc.sync.dma_start(out[isl, 0:HALF], dA[:])
                # dB stores ride SWDGE (GpSimd is otherwise idle): two
                # independent DMA paths instead of one FIFO HWDGE ring.
                nc.gpsimd.dma_start(out[isl, HALF:S], dB[:])

    return nc


_cached_nc = None


def _prep_inputs(matrix_1):
    """Host-side prep: fp8 cast, transpose, norms (tiny vs the S^2*R work)."""
    matrix_1 = np.asarray(matrix_1, dtype=np.float32)
    assert matrix_1.shape == (B, S, R)
    np8 = mybir.dt.np(F8)
    in_maps = []
    nis = []
    for b in range(B):
        x8 = matrix_1[b].astype(np8)
        xf = x8.astype(np.float32)
        ni = np.sum(xf * xf, axis=1)                      # [S] fp32
        nis.append(ni)
        in_maps.append({
            # [p, k, j] layout matching the XTD tile: xt[p,k,j] = x[j, 128k+p]
            "xt": np.ascontiguousarray(
                x8.T.reshape(KH, P, S).transpose(1, 0, 2)),
            "ni": np.ascontiguousarray(ni.reshape(NT, P).T),
        })
    return in_maps, np.stack(nis, axis=0)


def run(matrix_1, trace=False, tmpdir=None, **spmd_kwargs):
    """Run the SPMD kernel on 8 cores; returns (out [8,S,S] fp32, results)."""
    global _cached_nc
    if _cached_nc is None:
        _cached_nc = build_nc()
    nc = _cached_nc
    # The axon/PJRT path serializes nc as-is; Bacc's compile() (reg alloc,
    # matmul wait splitting) only runs inside finalize(), so do it here.
    if not nc.is_finalized():
        nc.finalize()
    in_maps, nis = _prep_inputs(matrix_1)
    try:
        res = run_bass_kernel_spmd(
            nc, in_maps, list(range(B)), tmpdir=tmpdir, trace=trace, **spmd_kwargs
        )
    except Exception:
        # transient device wedges (NRT_EXEC_UNIT_UNRECOVERABLE) clear on retry
        res = run_bass_kernel_spmd(
            nc, in_maps, list(range(B)), tmpdir=tmpdir, trace=trace, **spmd_kwargs
        )
    out = np.stack(
        [res.results[b]["out"].astype(np.float32) for b in range(B)], axis=0
    )
    # finish: +nj along the column axis, then the relu clamp
    out += nis[:, None, :]
    np.maximum(out, 0.0, out=out)
    return out, res


def kernel(matrix_1):
    out, _ = run(matrix_1)
    return out


# revision 18
# speedup vs baseline: 1.0414x; 1.0414x over previous
"""Pairwise squared-Euclidean distance kernel for TRN2 (8 NeuronCores).

Problem: matrix_1 [8, 2048, 256] fp32 -> out [8, 2048, 2048] fp32 with
  out[b,i,j] = max(||x_i||^2 + ||x_j||^2 - 2 x_i.x_j, 0)

Sharding: data-parallel over batch; core b handles matrix_1[b] entirely.

Host-side prep (per core, <0.1% of total FLOPs):
  xt [256, 2048] fp8(e4m3) = x.T   (both matmul operands)
  ni [128, 16]   fp32  row norms of the quantized x (per-partition bias)

Device plan per 128-row block i:
  4 fp8 DoubleRow matmuls (K=256 in ONE pass each; the two 128-row
  contraction chunks ride the [128, 2, n] interleaved APs) -> psum = G
  psA (banks 0-1) -> ACT:  dA = Identity(-2*psA + ni)       [fp16]
  psB (banks 2-3) -> DVE:  dB = (psB * -2) + ni  (2-scalar) [fp16]
  (separate tiles: two readers of one PSUM tile get serialized by Tile)
  2 stores of 256 KiB -> out rows

Host then finishes: out = max(out + nj, 0) in fp32. fp8 inputs cost
~2.5e-3 norm-rel (vs the 2e-2 gate); fp16 output adds ~3e-4. DoubleRow
halves the PE streaming passes (4 matmuls/block, not 8), and the split
ACT/DVE readers drain PSUM concurrently -- together they cut the block
cadence below the single-reader (1.9us) and 8-matmul (1.8us) floors.
~32 junk matmuls warm the PE's HAM clock gate during the input-DMA wait.
"""

import numpy as np

import concourse.bass as bass
import concourse.mybir as mybir
from concourse import bacc, tile
from concourse.bass_utils import run_bass_kernel_spmd

B, S, R = 8, 2048, 256
P = 128            # SBUF partitions
NT = S // P        # 16 row blocks
NBW = 512          # matmul moving-dim block = one fp32 PSUM bank
NB = S // NBW      # 4 col blocks
KH = R // P        # 2 contraction chunks
HALF = S // 2      # ACT/DVE reader split point
NWARM = 30         # HAM warm-up matmuls

F32 = mybir.dt.float32
F16 = mybir.dt.float16
F8 = mybir.dt.float8e4


def build_nc():
    # Bacc (not plain Bass): its compile() runs move_matmul_waits_to_ldweights
    # + generate_event_semaphores, without which walrus rejects matmuls that
    # accumulated >1 semaphore wait ("Too many sync wait commands").
    nc = bacc.Bacc()
    xt = nc.declare_dram_parameter("xt", [P, KH, S], F8, isOutput=False)
    ni_in = nc.declare_dram_parameter("ni", [P, NT], F32, isOutput=False)
    out = nc.declare_dram_parameter("out", [S, S], F16, isOutput=True)

    with tile.TileContext(nc) as tc:
        with (
            tc.tile_pool(name="const", bufs=1) as cpool,
            tc.tile_pool(name="xt", bufs=1) as xt_pool,
            tc.tile_pool(name="nrm", bufs=1) as nrm_pool,
            tc.tile_pool(name="obufA", bufs=4) as oA_pool,
            tc.tile_pool(name="obufB", bufs=4) as oB_pool,
            tc.tile_pool(name="psumA", bufs=2, space="PSUM") as psumA_pool,
            tc.tile_pool(name="psumB", bufs=2, space="PSUM") as psumB_pool,
        ):
            # HAM warm-up inputs: only need their memsets, no DMA dependency.
            junkw = cpool.tile([P, P], F16)
            junkm = cpool.tile([P, P], F16)
            nc.gpsimd.memset(junkw[:], 0.0)
            nc.gpsimd.memset(junkm[:], 0.0)

            # Both contraction chunks in one tile so a single [128, 2, n]
            # AP feeds DoubleRow: XTD[:, k, j] = x[j, 128k + p].
            XTD = xt_pool.tile([P, KH, S], F8)
            NI = nrm_pool.tile([P, NT], F32)

            # One 512 KiB load, issued FIRST: the host uploads xt already
            # in the tile's [p, k, j] layout (4 KiB contiguous/partition),
            # and everything downstream waits on it.
            nc.sync.dma_start(XTD[:], xt[:, :, :])
            nc.sync.dma_start(NI[:], ni_in[:, :])

            # Junk matmuls keep the PE continuously busy from t~6.5us until
            # the first data chunks land, so HAM unthrottles to 2.4 GHz.
            warmp = psumA_pool.tile([P, HALF], F32, tag="psA")
            for w in range(NWARM):
                nc.tensor.matmul(warmp[:, (w % 8) * P:(w % 8 + 1) * P],
                                 junkw[:], junkm[:], start=True, stop=True)

            # --- main loop over row blocks ---
            for i in range(NT):
                isl = slice(i * P, (i + 1) * P)
                psA = psumA_pool.tile([P, HALF], F32, tag="psA")
                psB = psumB_pool.tile([P, HALF], F32, tag="psB")
                lhsT = XTD[:, :, isl]
                for j in range(NB):
                    jsl = slice(j * NBW, (j + 1) * NBW)
                    dst = psA[:, jsl] if j < NB // 2 else \
                        psB[:, j * NBW - HALF:(j + 1) * NBW - HALF]
                    nc.tensor.matmul(
                        dst, lhsT, XTD[:, :, jsl],
                        start=True, stop=True,
                        perf_mode=mybir.MatmulPerfMode.DoubleRow,
                    )
                dA = oA_pool.tile([P, HALF], F16, tag="dA")
                dB = oB_pool.tile([P, HALF], F16, tag="dB")
                nc.scalar.activation(
                    dA[:], psA[:],
                    mybir.ActivationFunctionType.Identity,
                    bias=NI[:, i:i + 1], scale=-2.0,
                )
                nc.vector.tensor_scalar(
                    out=dB[:], in0=psB[:],
                    scalar1=-2.0, scalar2=NI[:, i:i + 1],
                    op0=mybir.AluOpType.mult, op1=mybir.AluOpType.add,
                )
                nc.sync.dma_start(out[isl, 0:HALF], dA[:])
                nc.sync.dma_start(out[isl, HALF:S], dB[:])

    return nc


_cached_nc = None


def _prep_inputs(matrix_1):
    """Host-side prep: fp8 cast, transpose, norms (tiny vs the S^2*R work)."""
    matrix_1 = np.asarray(matrix_1, dtype=np.float32)
    assert matrix_1.shape == (B, S, R)
    np8 = mybir.dt.np(F8)
    in_maps = []
    nis = []
    for b in range(B):
        x8 = matrix_1[b].astype(np8)
        xf = x8.astype(np.float32)
        ni = np.sum(xf * xf, axis=1)                      # [S] fp32
        nis.append(ni)
        in_maps.append({
            # [p, k, j] layout matching the XTD tile: xt[p,k,j] = x[j, 128k+p]
            "xt": np.ascontiguousarray(
                x8.T.reshape(KH, P, S).transpose(1, 0, 2)),
            "ni": np.ascontiguousarray(ni.reshape(NT, P).T),
        })
    return in_maps, np.stack(nis, axis=0)


def run(matrix_1, trace=False, tmpdir=None, **spmd_kwargs):
    """Run the SPMD kernel on 8 cores; returns (out [8,S,S] fp32, results)."""
    global _cached_nc
    if _cached_nc is None:
        _cached_nc = build_nc()
    nc = _cached_nc
    # The axon/PJRT path serializes nc as-is; Bacc's compile() (reg alloc,
    # matmul wait splitting) only runs inside finalize(), so do it here.
    if not nc.is_finalized():
        nc.finalize()
    in_maps, nis = _prep_inputs(matrix_1)
    try:
        res = run_bass_kernel_spmd(
            nc, in_maps, list(range(B)), tmpdir=tmpdir, trace=trace, **spmd_kwargs
        )
    except Exception:
        # transient device wedges (NRT_EXEC_UNIT_UNRECOVERABLE) clear on retry
        res = run_bass_kernel_spmd(
            nc, in_maps, list(range(B)), tmpdir=tmpdir, trace=trace, **spmd_kwargs
        )
    out = np.stack(
        [res.results[b]["out"].astype(np.float32) for b in range(B)], axis=0
    )
    # finish: +nj along the column axis, then the relu clamp
    out += nis[:, None, :]
    np.maximum(out, 0.0, out=out)
    return out, res


def kernel(matrix_1):
    out, _ = run(matrix_1)
    return out


# revision 19
# speedup vs baseline: 1.1822x; 1.1353x over previous
"""Pairwise squared-Euclidean distance kernel for TRN2 (8 NeuronCores).

Problem: matrix_1 [8, 2048, 256] fp32 -> out [8, 2048, 2048] fp32 with
  out[b,i,j] = max(||x_i||^2 + ||x_j||^2 - 2 x_i.x_j, 0)

Sharding: data-parallel over batch; core b handles matrix_1[b] entirely.

Host-side prep (per core, <0.1% of total FLOPs):
  xt [256, 2048] fp8(e4m3) = x.T   (both matmul operands)
  ni [128, 16]   fp32  row norms of the quantized x (per-partition bias)

Device plan per 128-row block i:
  4 fp8 DoubleRow matmuls (K=256 in ONE pass each; the two 128-row
  contraction chunks ride the [128, 2, n] interleaved APs) -> psum = G
  psA (banks 0-1) -> ACT:  dA = Identity(-2*psA + ni)       [fp16]
  psB (banks 2-3) -> DVE:  dB = (psB * -2) + ni  (2-scalar) [fp16]
  (separate tiles: two readers of one PSUM tile get serialized by Tile)
  2 stores of 256 KiB -> out rows

Host then finishes: out = max(out + nj, 0) in fp32. fp8 inputs cost
~2.5e-3 norm-rel (vs the 2e-2 gate); fp16 output adds ~3e-4. DoubleRow
halves the PE streaming passes (4 matmuls/block, not 8), and the split
ACT/DVE readers drain PSUM concurrently -- together they cut the block
cadence below the single-reader (1.9us) and 8-matmul (1.8us) floors.
~32 junk matmuls warm the PE's HAM clock gate during the input-DMA wait.
"""

import numpy as np

import concourse.bass as bass
import concourse.mybir as mybir
from concourse import bacc, tile
from concourse.bass_utils import run_bass_kernel_spmd

B, S, R = 8, 2048, 256
P = 128            # SBUF partitions
NT = S // P        # 16 row blocks
NBW = 512          # matmul moving-dim block = one fp32 PSUM bank
NB = S // NBW      # 4 col blocks
KH = R // P        # 2 contraction chunks
HALF = S // 2      # ACT/DVE reader split point
NWARM = 30         # HAM warm-up matmuls

F32 = mybir.dt.float32
F16 = mybir.dt.float16
F8 = mybir.dt.float8e4


def build_nc():
    # Bacc (not plain Bass): its compile() runs move_matmul_waits_to_ldweights
    # + generate_event_semaphores, without which walrus rejects matmuls that
    # accumulated >1 semaphore wait ("Too many sync wait commands").
    nc = bacc.Bacc()
    xt = nc.declare_dram_parameter("xt", [P, KH, S], F8, isOutput=False)
    ni_in = nc.declare_dram_parameter("ni", [P, NT], F32, isOutput=False)
    out = nc.declare_dram_parameter("out", [S, S], F16, isOutput=True)

    with tile.TileContext(nc) as tc:
        with (
            tc.tile_pool(name="const", bufs=1) as cpool,
            tc.tile_pool(name="xt", bufs=1) as xt_pool,
            tc.tile_pool(name="nrm", bufs=1) as nrm_pool,
            tc.tile_pool(name="obufA", bufs=4) as oA_pool,
            tc.tile_pool(name="obufB", bufs=4) as oB_pool,
            tc.tile_pool(name="psumA", bufs=4, space="PSUM") as psumA_pool,
            tc.tile_pool(name="psumB", bufs=4, space="PSUM") as psumB_pool,
        ):
            # HAM warm-up inputs: only need their memsets, no DMA dependency.
            junkw = cpool.tile([P, P], F16)
            junkm = cpool.tile([P, P], F16)
            nc.gpsimd.memset(junkw[:], 0.0)
            nc.gpsimd.memset(junkm[:], 0.0)

            # Both contraction chunks in one tile so a single [128, 2, n]
            # AP feeds DoubleRow: XTD[:, k, j] = x[j, 128k + p].
            XTD = xt_pool.tile([P, KH, S], F8)
            NI = nrm_pool.tile([P, NT], F32)

            # One 512 KiB load, issued FIRST: the host uploads xt already
            # in the tile's [p, k, j] layout (4 KiB contiguous/partition),
            # and everything downstream waits on it.
            nc.sync.dma_start(XTD[:], xt[:, :, :])
            nc.sync.dma_start(NI[:], ni_in[:, :])

            # Junk matmuls keep the PE continuously busy from t~6.5us until
            # the first data chunks land, so HAM unthrottles to 2.4 GHz.
            warmp = psumA_pool.tile([P, NBW], F32, tag="psA")
            for w in range(NWARM):
                nc.tensor.matmul(warmp[:, (w % 4) * P:(w % 4 + 1) * P],
                                 junkw[:], junkm[:], start=True, stop=True)

            # --- main loop over row blocks ---
            for i in range(NT):
                isl = slice(i * P, (i + 1) * P)
                # One PSUM bank per matmul, 4-deep pools: each reader op
                # waits on exactly one matmul and the deep rotation absorbs
                # the per-block release latency that paced the 2-buf layout.
                lhsT = XTD[:, :, isl]
                pss = []
                for j in range(NB):
                    jsl = slice(j * NBW, (j + 1) * NBW)
                    pool = psumA_pool if j < NB // 2 else psumB_pool
                    ps1 = pool.tile([P, NBW], F32,
                                    tag="psA" if j < NB // 2 else "psB")
                    nc.tensor.matmul(
                        ps1[:], lhsT, XTD[:, :, jsl],
                        start=True, stop=True,
                        perf_mode=mybir.MatmulPerfMode.DoubleRow,
                    )
                    pss.append(ps1)
                dA = oA_pool.tile([P, HALF], F16, tag="dA")
                dB = oB_pool.tile([P, HALF], F16, tag="dB")
                for h in range(2):
                    hsl = slice(h * NBW, (h + 1) * NBW)
                    nc.scalar.activation(
                        dA[:, hsl], pss[h][:],
                        mybir.ActivationFunctionType.Identity,
                        bias=NI[:, i:i + 1], scale=-2.0,
                    )
                    nc.vector.tensor_scalar(
                        out=dB[:, hsl], in0=pss[2 + h][:],
                        scalar1=-2.0, scalar2=NI[:, i:i + 1],
                        op0=mybir.AluOpType.mult, op1=mybir.AluOpType.add,
                    )
                nc.sync.dma_start(out[isl, 0:HALF], dA[:])
                nc.sync.dma_start(out[isl, HALF:S], dB[:])

    return nc


_cached_nc = None


def _prep_inputs(matrix_1):
    """Host-side prep: fp8 cast, transpose, norms (tiny vs the S^2*R work)."""
    matrix_1 = np.asarray(matrix_1, dtype=np.float32)
    assert matrix_1.shape == (B, S, R)
    np8 = mybir.dt.np(F8)
    in_maps = []
    nis = []
    for b in range(B):
        x8 = matrix_1[b].astype(np8)
        xf = x8.astype(np.float32)
        ni = np.sum(xf * xf, axis=1)                      # [S] fp32
        nis.append(ni)
        in_maps.append({
            # [p, k, j] layout matching the XTD tile: xt[p,k,j] = x[j, 128k+p]
            "xt": np.ascontiguousarray(
                x8.T.reshape(KH, P, S).transpose(1, 0, 2)),
            "ni": np.ascontiguousarray(ni.reshape(NT, P).T),
        })
    return in_maps, np.stack(nis, axis=0)


def run(matrix_1, trace=False, tmpdir=None, **spmd_kwargs):
    """Run the SPMD kernel on 8 cores; returns (out [8,S,S] fp32, results)."""
    global _cached_nc
    if _cached_nc is None:
        _cached_nc = build_nc()
    nc = _cached_nc
    # The axon/PJRT path serializes nc as-is; Bacc's compile() (reg alloc,
    # matmul wait splitting) only runs inside finalize(), so do it here.
    if not nc.is_finalized():
        nc.finalize()
    in_maps, nis = _prep_inputs(matrix_1)
    try:
        res = run_bass_kernel_spmd(
            nc, in_maps, list(range(B)), tmpdir=tmpdir, trace=trace, **spmd_kwargs
        )
    except Exception:
        # transient device wedges (NRT_EXEC_UNIT_UNRECOVERABLE) clear on retry
        res = run_bass_kernel_spmd(
            nc, in_maps, list(range(B)), tmpdir=tmpdir, trace=trace, **spmd_kwargs
        )
    out = np.stack(
        [res.results[b]["out"].astype(np.float32) for b in range(B)], axis=0
    )
    # finish: +nj along the column axis, then the relu clamp
    out += nis[:, None, :]
    np.maximum(out, 0.0, out=out)
    return out, res


def kernel(matrix_1):
    out, _ = run(matrix_1)
    return out
